# revision 1
# baseline (speedup 1.0000x reference)
"""BiLSTM (reference nn_CharBiGRU) Trainium2 Bass kernel.

Strategy:
  - 8 cores, batch-sharded (8 rows each); each core runs BOTH directions'
    LSTM scans interleaved (fwd over x, bwd over a host-rotated x_proc).
  - Host precomputes the per-batch time rotations (gathers) from mask
    lengths; the map s = (L-1-t) mod T is an involution, used on both the
    input and output sides of the backward scan.
  - Per step, gates for both dirs are computed as G[b, j] accumulated in
    PSUM via col-tiled matmuls: 4 PE column strips (one per gate i,f,o,g),
    stationary = h.T / x_t.T (8 cols each), streaming Wh.T / Wi.T.
    Bias enters as a K=1 matmul of a ones row.
  - Cell math runs on ACT (sigmoid/tanh) + DVE/GPSIMD elementwise with
    both dirs packed (fwd at free 0:512, bwd 512:1024).
  - h is recycled into stationary layout via PE transposes (4 per step).
"""

import numpy as np

B, T, D, H = 64, 512, 512, 512
G4 = 4 * H
NCORES = 8
BL = B // NCORES

_CACHE = {}


def build_kernel(T_steps=T, dtype_mm="float32"):
    import concourse.bass as bass
    import concourse.bacc as bacc
    import concourse.mybir as mybir
    from concourse.tile import TileContext
    from concourse.masks import make_identity

    fp32 = mybir.dt.float32
    AF = mybir.ActivationFunctionType

    # Bacc (not plain Bass): its compile() pass splits multi-waits into
    # event-semaphore chains and moves matmul waits onto LDWEIGHTS —
    # without it walrus rejects 2-wait matmuls ("Too many sync wait").
    nc = bacc.Bacc()
    xtt = nc.declare_dram_parameter("xtt", [2, T_steps, D, BL], fp32, isOutput=False)
    wht = nc.declare_dram_parameter("wht", [2, 4, 128, G4], fp32, isOutput=False)
    wit = nc.declare_dram_parameter("wit", [2, 4, 128, G4], fp32, isOutput=False)
    brow = nc.declare_dram_parameter("brow", [2, G4], fp32, isOutput=False)
    h0t = nc.declare_dram_parameter("h0t", [D, BL], fp32, isOutput=False)
    c0 = nc.declare_dram_parameter("c0", [BL, H], fp32, isOutput=False)
    ys = nc.declare_dram_parameter("ys", [2, T_steps, BL, H], fp32, isOutput=True)

    with TileContext(nc) as tc:
        with (
            tc.tile_pool(name="const", bufs=1) as constp,
            tc.tile_pool(name="wpool", bufs=1) as wpool,
            tc.tile_pool(name="state", bufs=1) as statep,
            tc.tile_pool(name="work", bufs=2) as workp,
            tc.tile_pool(name="xin", bufs=4) as xinp,
            tc.tile_pool(name="gpsum", bufs=2, space="PSUM") as psump,
            tc.tile_pool(name="ptpsum", bufs=2, space="PSUM") as ptp,
        ):
            ident = constp.tile([8, 8], fp32)
            make_identity(nc, ident[:, :])
            ones = constp.tile([1, 32], fp32)
            nc.gpsimd.memset(ones[:, :], 1.0)
            biasT = constp.tile([1, 2 * G4], fp32)
            for d in range(2):
                nc.sync.dma_start(out=biasT[0:1, d * G4:(d + 1) * G4], in_=brow[d:d + 1, :])

            # Weights in SBUF: one tile per (d, k) chunk = one DMA producer
            # each, so consuming matmuls carry a single sync-wait.
            whk = [[wpool.tile([128, G4], fp32, tag=f"wh{d}{k}", name=f"wh{d}{k}") for k in range(4)]
                   for d in range(2)]
            wik = [[wpool.tile([128, G4], fp32, tag=f"wi{d}{k}", name=f"wi{d}{k}") for k in range(4)]
                   for d in range(2)]
            for d in range(2):
                for k in range(4):
                    nc.sync.dma_start(out=whk[d][k][:, :], in_=wht[d, k])
                    nc.sync.dma_start(out=wik[d][k][:, :], in_=wit[d, k])

            # State: hT free = 16*k + 8*d + b ; c at base partition 32
            # (pairs with f-gate rows 32:40 in DVE tensor_tensor ops, which
            # require both SBUF inputs at the same base partition)
            hT = statep.tile([128, 64], fp32, tag="hT")
            C40 = statep.tile([40, 1024], fp32, tag="C40")
            c = C40[32:40, :]
            for k in range(4):
                nc.sync.dma_start(out=hT[:, 16 * k:16 * k + 8], in_=h0t[128 * k:128 * (k + 1), :])
                nc.sync.dma_start(out=hT[:, 16 * k + 8:16 * k + 16], in_=h0t[128 * k:128 * (k + 1), :])
            nc.sync.dma_start(out=c[:, 0:H], in_=c0[:, :])
            nc.sync.dma_start(out=c[:, H:2 * H], in_=c0[:, :])

            for t in range(T_steps):
                # x_t stationary tiles: one tile + one DMA per (d, k)
                xtk = [[xinp.tile([128, 8], fp32, tag=f"xt{d}{k}", name=f"xt{d}{k}") for k in range(4)]
                       for d in range(2)]
                for d in range(2):
                    for k in range(4):
                        nc.sync.dma_start(
                            out=xtk[d][k][:, :],
                            in_=xtt[d, t, 128 * k:128 * (k + 1), :],
                        )

                # Gates: G[32s + b, 512*d + jj] ; strip s = gate (i,f,o,g)
                G = psump.tile([128, 1024], fp32, tag="G")
                for d in range(2):
                    for s in range(4):
                        out_ap = G[32 * s:32 * s + 8, 512 * d:512 * (d + 1)]
                        tp = (0, 32 * s)
                        jo = d * G4 + 512 * s
                        # bias matmul writes the FULL 32-row strip (start=True)
                        # so no PSUM row is left uninitialized for the ACT reads
                        nc.tensor.matmul(
                            G[32 * s:32 * s + 32, 512 * d:512 * (d + 1)],
                            ones[0:1, 0:32], biasT[0:1, jo:jo + 512],
                            start=True, stop=False, tile_position=tp, skip_group_check=True,
                        )
                        for k in range(4):
                            nc.tensor.matmul(
                                out_ap, xtk[d][k][:, :],
                                wik[d][k][:, 512 * s:512 * s + 512],
                                start=False, stop=False, tile_position=tp, skip_group_check=True,
                            )
                        for k in range(4):
                            ho = 16 * k + 8 * d
                            nc.tensor.matmul(
                                out_ap, hT[:, ho:ho + 8],
                                whk[d][k][:, 512 * s:512 * s + 512],
                                start=False, stop=(k == 3), tile_position=tp, skip_group_check=True,
                            )

                # Activations: rows 0:96 = i,f,o -> sigmoid (i@0:8, f@32:40,
                # o@64:72); g -> tanh remapped to base 0 so it can pair with i
                A = workp.tile([96, 1024], fp32, tag="A")
                nc.scalar.activation(A[:, :], G[0:96, :], AF.Sigmoid)
                TG = workp.tile([8, 1024], fp32, tag="TG")
                nc.scalar.activation(TG[:, :], G[96:104, :], AF.Tanh)

                T1 = workp.tile([8, 1024], fp32, tag="T1")
                T2 = workp.tile([8, 1024], fp32, tag="T2")
                nc.vector.tensor_mul(T1[:, :], A[0:8, :], TG[:, :])       # bases 0,0
                nc.vector.tensor_mul(T2[:, :], A[32:40, :], C40[32:40, :])  # 32,32
                nc.vector.tensor_add(C40[32:40, :], T1[:, :], T2[:, :])   # out base 32
                TC = workp.tile([72, 1024], fp32, tag="TC")
                nc.scalar.activation(TC[64:72, :], C40[32:40, :], AF.Tanh)

                # h: fwd and bwd in separate base-0 tiles
                h2f = workp.tile([8, 512], fp32, tag="h2f")
                h2b = workp.tile([8, 512], fp32, tag="h2b")
                nc.vector.tensor_mul(h2f[:, :], A[64:72, 0:H], TC[64:72, 0:H])
                nc.gpsimd.tensor_mul(h2b[:, :], A[64:72, H:2 * H], TC[64:72, H:2 * H])

                nc.sync.dma_start(out=ys[0, t], in_=h2f[:, :])
                nc.sync.dma_start(out=ys[1, t], in_=h2b[:, :])

                # Recycle h into stationary layout: PT[:, 16k + 8d + b]
                PT = ptp.tile([128, 64], fp32, tag="PT")
                for k in range(4):
                    nc.tensor.transpose(
                        PT[:, 16 * k:16 * k + 8], h2f[:, 128 * k:128 * (k + 1)],
                        ident[:, :],
                    )
                    nc.tensor.transpose(
                        PT[:, 16 * k + 8:16 * k + 16], h2b[:, 128 * k:128 * (k + 1)],
                        ident[:, :],
                    )
                nc.vector.tensor_copy(hT[:, :], PT[:, :])

    nc.finalize()
    return nc


def _host_prep(inputs_emb, mask, h0, c0, Wi_f, Wh_f, b_f, Wi_b, Wh_b, b_b):
    x = np.asarray(inputs_emb, dtype=np.float32)
    mask = np.asarray(mask, dtype=np.float32)
    lengths = mask.astype(np.int32).sum(axis=1)  # [B]
    t_idx = np.arange(T, dtype=np.int64)[None, :]
    P = (lengths[:, None].astype(np.int64) - 1 - t_idx) % T  # [B, T] involution
    x_proc = np.take_along_axis(x, P[:, :, None], axis=1)  # [B, T, D]

    # xtt[d, t, :, b] layouts per core
    xtt_f = x.transpose(1, 2, 0)       # [T, D, B]
    xtt_b = x_proc.transpose(1, 2, 0)  # [T, D, B]

    # device strip order is (i, f, o, g); reference weights are (i, f, g, o)
    PERM = [0, 1, 3, 2]

    def chunks(W):
        # W: [4H, K] -> permute gate blocks -> W.T chunks [4, 128, 4H]
        W = np.asarray(W, dtype=np.float32)
        Wp = W.reshape(4, H, -1)[PERM].reshape(G4, -1)
        Wt = np.ascontiguousarray(Wp.T)  # [K, 4H]
        return Wt.reshape(4, 128, G4)

    def pbias(b):
        return np.asarray(b, np.float32).reshape(4, H)[PERM].reshape(G4)

    wht = np.stack([chunks(Wh_f), chunks(Wh_b)])  # [2, 4, 128, 4H]
    wit = np.stack([chunks(Wi_f), chunks(Wi_b)])
    brow = np.stack([pbias(b_f), pbias(b_b)])
    h0 = np.asarray(h0, np.float32)
    c0 = np.asarray(c0, np.float32)

    in_maps = []
    for cidx in range(NCORES):
        sl = slice(cidx * BL, (cidx + 1) * BL)
        in_maps.append({
            "xtt": np.ascontiguousarray(
                np.stack([xtt_f[:, :, sl], xtt_b[:, :, sl]])),
            "wht": wht, "wit": wit, "brow": brow,
            "h0t": np.ascontiguousarray(h0[sl].T),
            "c0": np.ascontiguousarray(c0[sl]),
        })
    return in_maps, P


def _host_post(results, P):
    ys_f = np.concatenate([r["ys"][0].transpose(1, 0, 2) for r in results], 0)  # [B,T,H]
    ys_b = np.concatenate([r["ys"][1].transpose(1, 0, 2) for r in results], 0)
    out_b = np.take_along_axis(ys_b, P[:, :, None], axis=1)
    return np.concatenate([ys_f, out_b], axis=-1).astype(np.float32)


def kernel(**inputs):
    from concourse.bass_utils import run_bass_kernel_spmd
    in_maps, P = _host_prep(**inputs)
    if "nc" not in _CACHE:
        _CACHE["nc"] = build_kernel()
    nc = _CACHE["nc"]
    res = run_bass_kernel_spmd(nc, in_maps, list(range(NCORES)))
    return _host_post(res.results, P)



# revision 13
# speedup vs baseline: 8.3627x; 8.3627x over previous
"""BiLSTM (reference nn_CharBiGRU) Trainium2 Bass kernel, v2.

Strategy (vs v1 which ran everything fp32 with x@Wi in-loop, ~35ms):
  - Shard: 2 directions x 4-way batch (BL=16 rows/core). Core c: dir=c//4,
    batch rows 16*(c%4). No cross-core communication (collective floors
    ~5us/call kill per-step schemes).
  - Phase 1 (GEMM): XW[t] = x_t @ Wi.T + b precomputed for all T as one
    large bf16 matmul (1 cyc/row vs fp32's 4), written to internal DRAM
    [128, T, 256] (q-major so PSUM->DRAM writes are AP-order compatible).
  - Phase 2 (recurrence): per step only h @ Wh.T remains. Weight-stationary
    bf16 tiles (FWL halves LDWEIGHTS); out lands H-major [gate-chunk 128,
    batch 16] so cell math runs full-partition-width and h needs NO
    transpose to become next step's moving operand.
  - Gate layout: m = 4*gt + u, gt in (i,f,o,g) [ref order i,f,g,o], u =
    H-chunk. PSUM G [128, 64*gt+16*u+b], accumulation groups kept
    CONSECUTIVE (k-inner; splitting a group across interleaved phases
    miscomputes on HW). Cell math at u-pair granularity ([128,32] DVE /
    [128,96] ACT slices) so ACT's ~185ns/op fixed cost amortizes; the
    pair-0 chain overlaps the pair-1 MMs, h ping-pongs per step.
  - bf16 weights/x/h with fp32 PSUM + fp32 cell state: host-simulated
    rel err ~5e-3 (gate is 2e-2).
"""

import numpy as np

B, T, D, H = 64, 512, 512, 512
G4 = 4 * H
NCORES = 8
BL = 16           # batch rows per core
NDIR_CORES = 4    # cores per direction

_CACHE = {}


def build_kernel(T_steps=T):
    import concourse.bass as bass
    import concourse.bacc as bacc
    import concourse.mybir as mybir
    from concourse.tile import TileContext

    fp32 = mybir.dt.float32
    bf16 = mybir.dt.bfloat16
    AF = mybir.ActivationFunctionType

    NBT = T_steps * BL
    assert NBT % 512 == 0, "T_steps must be a multiple of 32"
    NCH = NBT // 512  # GEMM n-chunks of 512

    nc = bacc.Bacc()
    xT = nc.declare_dram_parameter("xT", [4, 128, NBT], bf16, isOutput=False)
    wiT = nc.declare_dram_parameter("wiT", [4, 16, 128, 128], bf16, isOutput=False)
    whT = nc.declare_dram_parameter("whT", [4, 16, 128, 128], bf16, isOutput=False)
    biasT = nc.declare_dram_parameter("biasT", [1, G4], bf16, isOutput=False)
    h0T = nc.declare_dram_parameter("h0T", [128, 64], bf16, isOutput=False)
    c0T = nc.declare_dram_parameter("c0T", [128, 64], fp32, isOutput=False)
    ys = nc.declare_dram_parameter("ys", [T_steps, 128, 64], bf16, isOutput=True)
    # internal scratch: XW pre-activations, q-major [p, t, 64*gt+16*u+b]
    xw = nc.dram_tensor("xw", [128, T_steps, 256], fp32, kind="Internal")

    with TileContext(nc) as tc:
        with (
            tc.tile_pool(name="const", bufs=1) as constp,
            tc.tile_pool(name="wpool", bufs=1) as wpool,
            tc.tile_pool(name="state", bufs=1) as statep,
            tc.tile_pool(name="work", bufs=2) as workp,
            tc.tile_pool(name="xin", bufs=4) as xinp,
            tc.tile_pool(name="gout", bufs=4) as goutp,
            tc.tile_pool(name="gpsum", bufs=6, space="PSUM") as gpsump,
            tc.tile_pool(name="rpsum", bufs=2, space="PSUM") as rpsump,
        ):
            ones = constp.tile([1, 512], bf16)
            nc.gpsimd.memset(ones[:, :], 1.0)
            bias_sb = constp.tile([1, G4], bf16)
            nc.sync.dma_start(out=bias_sb[:, :], in_=biasT[:, :])

            # weights: one tile per (k, m) chunk, single DMA producer each
            wi_sb = [[wpool.tile([128, 128], bf16, tag=f"wi{k}_{m}", name=f"wi{k}_{m}") for m in range(16)]
                     for k in range(4)]
            wh_sb = [[wpool.tile([128, 128], bf16, tag=f"wh{k}_{m}", name=f"wh{k}_{m}") for m in range(16)]
                     for k in range(4)]
            for k in range(4):
                for m in range(16):
                    nc.sync.dma_start(out=wi_sb[k][m][:, :], in_=wiT[k, m])
                    nc.sync.dma_start(out=wh_sb[k][m][:, :], in_=whT[k, m])

            # x.T resident in SBUF for the GEMM
            xT_sb = [wpool.tile([128, NBT], bf16, tag=f"xT{k}", name=f"xT{k}") for k in range(4)]
            for k in range(4):
                nc.sync.dma_start(out=xT_sb[k][:, :], in_=xT[k])

            # persistent state; h ping-pongs per step so a step's matmuls
            # (reading h_{t-1}) never alias the cell math's h_t writes
            hT = [statep.tile([128, 64], bf16, tag=f"hT{j}", name=f"hT{j}")
                  for j in range(2)]
            cT = statep.tile([128, 64], fp32, tag="cT")
            nc.sync.dma_start(out=hT[0][:, :], in_=h0T[:, :])
            nc.sync.dma_start(out=cT[:, :], in_=c0T[:, :])

            # ---- Phase 1: XW GEMM ----
            for m in range(16):
                gt, u = m // 4, m % 4
                for nch in range(NCH):
                    ps = gpsump.tile([128, 512], fp32, tag="g")
                    nc.tensor.matmul(
                        ps[:, :], bias_sb[0:1, 128 * m:128 * m + 128],
                        ones[0:1, :], start=True, stop=False,
                        skip_group_check=True,
                    )
                    for k in range(4):
                        nc.tensor.matmul(
                            ps[:, :], wi_sb[k][m][:, :],
                            xT_sb[k][:, 512 * nch:512 * nch + 512],
                            start=False, stop=(k == 3), skip_group_check=True,
                        )
                    # psum -> sbuf staging (DMA cannot read PSUM), engines
                    # round-robined; all overlap the PE stream
                    gsb = goutp.tile([128, 512], fp32, tag="gsb")
                    if (m * NCH + nch) % 2 == 0:
                        nc.scalar.copy(gsb[:, :], ps[:, :])
                    else:
                        nc.vector.tensor_copy(gsb[:, :], ps[:, :])
                    # sbuf [q, (ti, b)] -> xw[q, 32*nch+ti, 64*gt+16*u+b]
                    nc.sync.dma_start(
                        out=xw[:, 32 * nch:32 * nch + 32,
                               64 * gt + 16 * u:64 * gt + 16 * u + 16],
                        in_=gsb[:, :],
                    )

            # ---- Phase 2: recurrence ----
            for t in range(T_steps):
                xwt = xinp.tile([128, 256], fp32, tag="xw")
                nc.sync.dma_start(out=xwt[:, :], in_=xw[:, t, :])

                G = rpsump.tile([128, 256], fp32, tag="G")
                S = workp.tile([128, 256], fp32, tag="S")
                hsrc, hdst = hT[t % 2], hT[(t + 1) % 2]
                # PE: per u-pair, consecutive k-inner accumulation groups
                for P in range(2):
                    for u in (2 * P, 2 * P + 1):
                        for gt in range(4):
                            m = 4 * gt + u
                            for k in range(4):
                                nc.tensor.matmul(
                                    G[:, 64 * gt + 16 * u:64 * gt + 16 * u + 16],
                                    wh_sb[k][m][:, :], hsrc[:, 16 * k:16 * k + 16],
                                    start=(k == 0), stop=(k == 3),
                                    skip_group_check=True,
                                )
                    # cell math for pair P (units 256P : 256P+256)
                    # A/S layout: [128, 128*P + 32*gt + 16*(u%2) + b]
                    for gt in range(4):
                        # PSUM-reading ops must stay off GPSIMD
                        nc.vector.tensor_add(
                            S[:, 128 * P + 32 * gt:128 * P + 32 * gt + 32],
                            G[:, 64 * gt + 32 * P:64 * gt + 32 * P + 32],
                            xwt[:, 64 * gt + 32 * P:64 * gt + 32 * P + 32],
                        )
                    nc.scalar.activation(
                        S[:, 128 * P:128 * P + 96], S[:, 128 * P:128 * P + 96],
                        AF.Sigmoid)
                    nc.scalar.activation(
                        S[:, 128 * P + 96:128 * P + 128],
                        S[:, 128 * P + 96:128 * P + 128], AF.Tanh)
                    T1 = workp.tile([128, 32], fp32, tag=f"t1_{P}")
                    T2 = workp.tile([128, 32], fp32, tag=f"t2_{P}")
                    TC = workp.tile([128, 32], fp32, tag=f"tc_{P}")
                    nc.vector.tensor_mul(
                        T1[:, :], S[:, 128 * P:128 * P + 32],
                        S[:, 128 * P + 96:128 * P + 128])          # si*tg
                    nc.vector.tensor_mul(
                        T2[:, :], S[:, 128 * P + 32:128 * P + 64],
                        cT[:, 32 * P:32 * P + 32])                 # sf*c
                    nc.vector.tensor_add(cT[:, 32 * P:32 * P + 32], T1[:, :], T2[:, :])
                    nc.scalar.activation(TC[:, :], cT[:, 32 * P:32 * P + 32], AF.Tanh)
                    nc.gpsimd.tensor_mul(
                        hdst[:, 32 * P:32 * P + 32],
                        S[:, 128 * P + 64:128 * P + 96], TC[:, :])  # so*tc -> bf16
                    nc.sync.dma_start(
                        out=ys[t, :, 32 * P:32 * P + 32],
                        in_=hdst[:, 32 * P:32 * P + 32])

    nc.finalize()
    return nc


def _host_prep(inputs_emb, mask, h0, c0, Wi_f, Wh_f, b_f, Wi_b, Wh_b, b_b):
    import ml_dtypes
    bf16 = ml_dtypes.bfloat16

    x = np.asarray(inputs_emb, dtype=np.float32)
    mask = np.asarray(mask, dtype=np.float32)
    lengths = mask.astype(np.int32).sum(axis=1)  # [B]
    t_idx = np.arange(T, dtype=np.int64)[None, :]
    P = (lengths[:, None].astype(np.int64) - 1 - t_idx) % T  # [B, T] involution
    x_proc = np.take_along_axis(x, P[:, :, None], axis=1)  # [B, T, D]

    # device gate-type order (i, f, o, g); reference rows are (i, f, g, o)
    PERM = [0, 1, 3, 2]

    def wtiles(W):
        # W [4H, K] -> permute gate blocks -> tiles [4k, 16m, 128p, 128q]
        W = np.asarray(W, dtype=np.float32)
        Wp = W.reshape(4, H, -1)[PERM].reshape(G4, -1)   # rows: m-chunks of 128
        Wt = np.ascontiguousarray(Wp.T)                  # [K, 4H]
        t4 = Wt.reshape(4, 128, 16, 128).transpose(0, 2, 1, 3)
        return np.ascontiguousarray(t4).astype(bf16)

    def pbias(b):
        return np.asarray(b, np.float32).reshape(4, H)[PERM].reshape(1, G4).astype(bf16)

    h0 = np.asarray(h0, np.float32)
    c0 = np.asarray(c0, np.float32)

    wis = [wtiles(Wi_f), wtiles(Wi_b)]
    whs = [wtiles(Wh_f), wtiles(Wh_b)]
    bs = [pbias(b_f), pbias(b_b)]
    xs = [x, x_proc]

    def state_t(s):  # [BL, H] -> [128, 64]: out[p, 16u+b] = s[b, 128u+p]
        return np.ascontiguousarray(s.reshape(BL, 4, 128).transpose(2, 1, 0).reshape(128, 64))

    in_maps = []
    for cidx in range(NCORES):
        d = cidx // NDIR_CORES
        sl = slice(BL * (cidx % NDIR_CORES), BL * (cidx % NDIR_CORES) + BL)
        xp = xs[d][sl]  # [BL, T, D]
        # xT[k, p, t*16+b] = xp[b, t, 128k+p]
        xTc = xp.transpose(2, 1, 0).reshape(4, 128, T * BL).astype(bf16)
        in_maps.append({
            "xT": np.ascontiguousarray(xTc),
            "wiT": wis[d], "whT": whs[d], "biasT": bs[d],
            "h0T": state_t(h0[sl]).astype(bf16),
            "c0T": state_t(c0[sl]),
        })
    return in_maps, P


def _host_post(results, P):
    # ys [T, 128, 64] bf16 -> [BL, T, H]: out[b, t, 128u+p] = ys[t, p, 16u+b]
    def core_out(ys_c):
        y = np.asarray(ys_c).astype(np.float32)
        return y.reshape(T, 128, 4, BL).transpose(3, 0, 2, 1).reshape(BL, T, H)

    ys_f = np.concatenate([core_out(results[c]["ys"]) for c in range(4)], 0)
    ys_b = np.concatenate([core_out(results[c]["ys"]) for c in range(4, 8)], 0)
    out_b = np.take_along_axis(ys_b, P[:, :, None], axis=1)
    return np.concatenate([ys_f, out_b], axis=-1).astype(np.float32)


def kernel(**inputs):
    from concourse.bass_utils import run_bass_kernel_spmd
    in_maps, P = _host_prep(**inputs)
    if "nc" not in _CACHE:
        _CACHE["nc"] = build_kernel()
    nc = _CACHE["nc"]
    res = run_bass_kernel_spmd(nc, in_maps, list(range(NCORES)))
    return _host_post(res.results, P)
